# revision 30
# baseline (speedup 1.0000x reference)
"""Trainium2 Bass kernel for nn_AttentionLayer (dense transformer block with
summed heads), distributed over 8 NeuronCores.

Sharding: 4 batches x 2 head-groups (8 heads each), as the baseline — but
restructured for speed:
  - bf16 matmul datapath (fp32 PSUM), halving DMA + LDWEIGHTS traffic.
  - dc-outer projections with 8 PSUM accumulators so the first matmul only
    needs the first 128-row chunk of x/W (input DMA overlaps compute).
  - causal restriction: score/z matmuls only cover columns >= the block
    diagonal, so fully-masked regions are never computed and the softmax
    denominator is exact without additive -1e11 masking; only the diagonal
    128x128 triangle needs a 0/1 multiply.
  - softmax normalize: denominators for all 8 heads are batched into ONE
    vector reciprocal, broadcast across partitions with a one-hot matmul on
    the PE, and folded with a short tensor-tensor tree (replaces 16 x 3.3us
    reciprocals + 16 gpsimd broadcasts).
  - sequence-half-outer loop with one pairwise AllReduce per half: the
    first collective overlaps the second half's attention, and the first
    half's ff Dense runs during the second half too. Both cores of a pair
    compute the full [S, D] output (identical after AllReduce), host takes
    one copy.
"""

import sys

sys.path.insert(0, "/opt/trn_rl_repo")

import numpy as np
import ml_dtypes

import concourse.bass as bass
import concourse.bacc as bacc
import concourse.mybir as mybir
import concourse.tile as tile
from concourse.bass_utils import run_bass_kernel_spmd

B, S, D, H, DH = 4, 1024, 1024, 16, 64
HL, NPAIR = 8, 4          # heads / head-pairs per core
FP32 = mybir.dt.float32
F32R = mybir.dt.float32r
BF16 = mybir.dt.bfloat16
AF = mybir.ActivationFunctionType
ALU = mybir.AluOpType
RG = [[0, 1], [2, 3], [4, 5], [6, 7]]
BF = ml_dtypes.bfloat16


def _r(ap):
    return ap.bitcast(F32R)


def build_nc():
    nc = bacc.Bacc("TRN2", target_bir_lowering=False, num_devices=8)

    xT = nc.declare_dram_parameter("xT", [D, S], BF16, isOutput=False)
    wq = nc.declare_dram_parameter("wq", [D, HL * DH], BF16, isOutput=False)
    wk = nc.declare_dram_parameter("wk", [D, HL * DH], BF16, isOutput=False)
    wv = nc.declare_dram_parameter("wv", [D, HL * DH], BF16, isOutput=False)
    bqk = nc.declare_dram_parameter("bqk", [128, 2 * NPAIR], FP32, isOutput=False)
    bvt = nc.declare_dram_parameter("bvt", [128, HL * DH], FP32, isOutput=False)
    tri01 = nc.declare_dram_parameter("tri01", [128, 128], BF16, isOutput=False)
    onesp = nc.declare_dram_parameter("onesp", [128, 8, HL], BF16, isOutput=False)
    selp = nc.declare_dram_parameter("selp", [8, NPAIR, 128], FP32, isOutput=False)
    foldp = nc.declare_dram_parameter("foldp", [128, DH], FP32, isOutput=False)
    wfa = nc.declare_dram_parameter("wfa", [DH + 1, D], BF16, isOutput=False)
    onesr = nc.declare_dram_parameter("onesr", [1, 512], BF16, isOutput=False)
    out_ext = nc.declare_dram_parameter("out", [S, D], FP32, isOutput=True)
    import os
    DBG = os.environ.get("KDBG") == "1"
    if DBG:
        dq = nc.declare_dram_parameter("dq", [128, S], BF16, isOutput=True)
        dv = nc.declare_dram_parameter("dv", [128, HL * (DH + 1)], BF16, isOutput=True)
        dnum = nc.declare_dram_parameter("dnum", [128, S], BF16, isOutput=True)
        dden = nc.declare_dram_parameter("dden", [HL, S], FP32, isOutput=True)
        dzs = nc.declare_dram_parameter("dzs", [DH, S], FP32, isOutput=True)
        drs = nc.declare_dram_parameter("drs", [DH + 1, 512], FP32, isOutput=True)
        drp = nc.declare_dram_parameter("drp", [128, S], FP32, isOutput=True)
        dsg = nc.declare_dram_parameter("dsg", [1, HL, S], FP32, isOutput=True)
        dtm = nc.declare_dram_parameter("dtm", [128, 512], FP32, isOutput=True)

    with tile.TileContext(nc) as tc:
        with (
            tc.tile_pool(name="const", bufs=1) as constp,
            tc.tile_pool(name="qkv", bufs=1) as qkvp,
            tc.tile_pool(name="dram", bufs=1, space="DRAM") as dramp,
            tc.tile_pool(name="w0pool", bufs=9) as w0pool,
            tc.tile_pool(name="outp", bufs=3) as outp,
        ):
            # ---- persistent activations ----
            qT = qkvp.tile([128, NPAIR, S], BF16)   # [(2 heads)*64e, pair, q]
            kT = qkvp.tile([128, NPAIR, S], BF16)
            vA = qkvp.tile([128, 8, HL, DH + 1], BF16)  # [t_in, t_chunk, head, e|1]
            numT = qkvp.tile([128, NPAIR, S], BF16)  # normalized-pending z numerators
            denT = qkvp.tile([128, S], FP32)
            stageD = qkvp.tile([1, HL, S], FP32)
            recipT = qkvp.tile([128, S], F32R)
            zsum = qkvp.tile([DH, S], BF16)
            rs0 = qkvp.tile([DH + 1, 512], BF16)
            rs1 = qkvp.tile([DH + 1, 512], BF16)

            # ---- constants (emitted early; small) ----
            bqk_sb = constp.tile([128, 2 * NPAIR], FP32)
            bvt_sb = constp.tile([128, HL * DH], FP32)
            tri_sb = constp.tile([128, 128], BF16)
            sel_sb = constp.tile([8, NPAIR, 128], F32R)
            fold_sb = constp.tile([128, DH], F32R)
            wfa_sb = constp.tile([DH + 1, D], BF16)

            # ---- phase 1: projections (dc-outer, 8 PSUM accumulators) ----
            with (
                tc.tile_pool(name="xp", bufs=8) as xp,
                tc.tile_pool(name="wtp", bufs=3) as wtp,
                tc.tile_pool(name="projp", bufs=8, space="PSUM") as projp,
            ):
                xts = []
                wq_sb = wtp.tile([128, 8, HL * DH], BF16, tag="w")
                wk_sb = wtp.tile([128, 8, HL * DH], BF16, tag="w")
                wv_sb = wtp.tile([128, 8, HL * DH], BF16, tag="w")
                for dc in range(8):
                    t = xp.tile([128, S], BF16, tag="xT")
                    nc.sync.dma_start(t[:], xT[dc * 128:(dc + 1) * 128, :])
                    xts.append(t)
                    nc.scalar.dma_start(wq_sb[:, dc, :], wq[dc * 128:(dc + 1) * 128, :])
                for dc in range(8):
                    nc.scalar.dma_start(wk_sb[:, dc, :], wk[dc * 128:(dc + 1) * 128, :])
                for dc in range(8):
                    nc.scalar.dma_start(wv_sb[:, dc, :], wv[dc * 128:(dc + 1) * 128, :])
                nc.scalar.dma_start(bqk_sb[:], bqk[:])
                nc.scalar.dma_start(bvt_sb[:], bvt[:])
                nc.scalar.dma_start(tri_sb[:], tri01[:])
                nc.scalar.dma_start(vA[:, :, :, DH:DH + 1], onesp[:])
                nc.scalar.dma_start(sel_sb[:], selp[:].bitcast(F32R))
                nc.scalar.dma_start(fold_sb[:], foldp[:].bitcast(F32R))
                nc.scalar.dma_start(wfa_sb[:], wfa[:])

                # q then k: 8 live accumulators each, contraction (dc) outer
                for w_sb, dst, boff in ((wq_sb, qT, 0), (wk_sb, kT, NPAIR)):
                    pss = [projp.tile([128, 512], FP32, tag="pp", name=f"pp{i}") for i in range(8)]
                    for dc in range(8):
                        for i in range(8):
                            p, s2 = i // 2, i % 2
                            nc.tensor.matmul(
                                pss[i][:],
                                w_sb[:, dc, p * 128:(p + 1) * 128],
                                xts[dc][:, s2 * 512:(s2 + 1) * 512],
                                start=(dc == 0), stop=(dc == 7),
                            )
                    for i in range(8):
                        p, s2 = i // 2, i % 2
                        nc.vector.tensor_scalar_add(
                            dst[:, p, s2 * 512:(s2 + 1) * 512], pss[i][:],
                            bqk_sb[:, boff + p:boff + p + 1])

                # v: out rows = key chunks, two 4-slot passes with the first
                # sequence-half's score/exp stream interleaved between them
                def v_pass(lo4):
                    pss = [projp.tile([128, 512], FP32, tag="pp", name=f"pv{i}")
                           for i in range(4)]
                    for dc in range(8):
                        for i in range(4):
                            t_c = lo4 + i
                            nc.tensor.matmul(
                                pss[i][:],
                                xts[dc][:, t_c * 128:(t_c + 1) * 128],
                                wv_sb[:, dc, :],
                                start=(dc == 0), stop=(dc == 7),
                            )
                    for i in range(4):
                        t_c = lo4 + i
                        nc.vector.tensor_tensor(
                            vA[:, t_c, :, :DH],
                            pss[i][:].rearrange("p (h e) -> p h e", h=HL),
                            bvt_sb[:].rearrange("p (h e) -> p h e", h=HL),
                            ALU.add,
                        )

                def sc0_scores(p, hh):
                    rows = slice(hh * 64, hh * 64 + 64)
                    wT = w0pool.tile([128, 4, 512], BF16, tag="wT0", name="wT0")
                    for t_c in range(4):
                        lo = 128 * t_c
                        ps = projp.tile([128, 512], FP32, tag="pp", name="ps0")
                        nc.tensor.matmul(
                            ps[:, lo:],
                            kT[rows, p, t_c * 128:(t_c + 1) * 128],
                            qT[rows, p, lo:512],
                            start=True, stop=True,
                        )
                        nc.scalar.activation(
                            wT[:, t_c, lo:], ps[:, lo:], AF.Exp, scale=0.125)
                        nc.vector.tensor_tensor(
                            wT[:, t_c, lo:lo + 128],
                            wT[:, t_c, lo:lo + 128],
                            tri_sb[:], ALU.mult)
                    return wT

                w0s = []
                v_pass(0)
                w0s.append(sc0_scores(0, 0))
                w0s.append(sc0_scores(0, 1))
                v_pass(4)
                for p in range(NPAIR):
                    for hh in range(2):
                        if p == 0:
                            continue
                        w0s.append(sc0_scores(p, hh))

            # ---- phases 2-4: attention / normalize / collectives / ff ----
            with (
                tc.tile_pool(name="spsum", bufs=3, space="PSUM") as spsum,
                tc.tile_pool(name="zpsum", bufs=3, space="PSUM") as zpsum,
                tc.tile_pool(name="bcpsum", bufs=2, space="PSUM") as bcpsum,
                tc.tile_pool(name="wpool", bufs=4) as wpool,
                tc.tile_pool(name="smallp", bufs=3) as smallp,
            ):
                zouts = []
                nc.vector.memset(denT[:], 1.0)

                def ff_half(half, rs_t):
                    # rs_t rows 0..63 = allreduced z, row 64 = 1 (bias row of wfa)
                    nc.sync.dma_start(rs_t[:DH, :], zouts[half][:])
                    nc.sync.dma_start(rs_t[DH:DH + 1, :], onesr[:])
                    for j in range(4):
                        for dcol in range(2):
                            dsl = slice(dcol * 512, (dcol + 1) * 512)
                            fps = zpsum.tile([128, 512], FP32, tag="zp")
                            nc.tensor.matmul(
                                fps[:],
                                rs_t[:, j * 128:(j + 1) * 128],
                                wfa_sb[:, dsl],
                                start=True, stop=True,
                            )
                            og = outp.tile([128, 512], FP32, tag="og")
                            nc.scalar.activation(og[:], fps[:], AF.Gelu)
                            eng = nc.sync if (j + dcol) % 2 == 0 else nc.scalar
                            eng.dma_start(
                                out_ext[half * 512 + j * 128:half * 512 + (j + 1) * 128, dsl],
                                og[:])

                def emit_scores(sc, p, hh):
                    C = 4 * sc + 4
                    rows = slice(hh * 64, hh * 64 + 64)
                    wT = wpool.tile([128, 8, 512], BF16, tag="wT", name="wT")
                    for t_c in range(C):
                        lo = 128 * (t_c - 4 * sc) if t_c >= 4 * sc else 0
                        ps = spsum.tile([128, 512], FP32, tag="sp", name="ps")
                        nc.tensor.matmul(
                            ps[:, lo:],
                            kT[rows, p, t_c * 128:(t_c + 1) * 128],
                            qT[rows, p, sc * 512 + lo:(sc + 1) * 512],
                            start=True, stop=True,
                        )
                        nc.scalar.activation(
                            wT[:, t_c, lo:], ps[:, lo:], AF.Exp, scale=0.125)
                        if t_c >= 4 * sc:
                            nc.vector.tensor_tensor(
                                wT[:, t_c, lo:lo + 128],
                                wT[:, t_c, lo:lo + 128],
                                tri_sb[:], ALU.mult)
                    return wT

                def emit_z(sc, p, hh, wT):
                    C = 4 * sc + 4
                    ssl = slice(sc * 512, (sc + 1) * 512)
                    rows = slice(hh * 64, hh * 64 + 64)
                    h_loc = 2 * p + hh
                    zaug = zpsum.tile([128, 512], FP32, tag="zp", name="zaug")
                    for t_c in range(C):
                        lo = 128 * (t_c - 4 * sc) if t_c >= 4 * sc else 0
                        nc.tensor.matmul(
                            zaug[:DH + 1, lo:],
                            vA[:, t_c, h_loc, :],
                            wT[:, t_c, lo:],
                            start=(t_c == 0), stop=(t_c == C - 1),
                            skip_group_check=True,
                        )
                    nc.vector.tensor_copy(
                        stageD[0:1, h_loc, ssl], zaug[DH:DH + 1, :])
                    nc.vector.tensor_copy(
                        numT[rows, p, ssl], zaug[:DH, :])

                def den_stage_a(sc):
                    ssl = slice(sc * 512, (sc + 1) * 512)
                    t = dramp.tile([6, 512], FP32, name=f"ddena{sc}")
                    nc.sync.dma_start(t[:], stageD[0:1, 0:6, ssl])
                    nc.sync.dma_start(denT[0:6, ssl], t[:])

                def normalize_cc(sc):
                    ssl = slice(sc * 512, (sc + 1) * 512)
                    dden_b = dramp.tile([2, 512], FP32, name=f"ddenb{sc}")
                    nc.sync.dma_start(dden_b[:], stageD[0:1, 6:8, ssl])
                    nc.sync.dma_start(denT[6:8, ssl], dden_b[:])
                    with nc.allow_low_precision(reason="f32r is fp32 bits"):
                        nc.vector.reciprocal(recipT[:, ssl], denT[:, ssl])
                    tmps = []
                    for p in range(NPAIR):
                        bc = bcpsum.tile([128, 512], FP32, tag="bc")
                        nc.tensor.matmul(
                            bc[:], sel_sb[:, p, :], recipT[:HL, ssl],
                            start=True, stop=True,
                        )
                        tmp = smallp.tile([128, 512], F32R, tag=f"tmp{p}")
                        nc.vector.tensor_tensor(
                            tmp[:], numT[:, p, ssl], bc[:], ALU.mult)
                        tmps.append(tmp)
                    zps = bcpsum.tile([DH, 512], FP32, tag="bc", name="zps")
                    for p in range(NPAIR):
                        nc.tensor.matmul(
                            zps[:], fold_sb[:], tmps[p][:],
                            start=(p == 0), stop=(p == NPAIR - 1),
                        )
                    nc.vector.tensor_copy(zsum[:, ssl], zps[:])
                    zin = dramp.tile([DH, 512], BF16, name=f"zin{sc}")
                    nc.sync.dma_start(zin[:], zsum[:, ssl])
                    zout = dramp.tile([DH, 512], BF16, name=f"zout{sc}")
                    nc.gpsimd.collective_compute(
                        "AllReduce", ALU.add, replica_groups=RG,
                        ins=[zin[:].opt()], outs=[zout[:].opt()],
                    )
                    zouts.append(zout)

                GRP = [(p, hh) for p in range(NPAIR) for hh in range(2)]
                # first half: consume prefetched scores; interleave the second
                # half's first score groups so the exp stream never goes cold
                # across the first half's z-chains and CC0's issue window
                sc1_pend = []
                for gi, (p, hh) in enumerate(GRP):
                    emit_z(0, p, hh, w0s[gi])
                    if (p, hh) == (2, 1):
                        den_stage_a(0)
                    if gi in (1, 3, 5):
                        g = GRP[len(sc1_pend)]
                        sc1_pend.append((g[0], g[1], emit_scores(1, *g)))
                normalize_cc(0)
                for (p, hh) in GRP[len(sc1_pend):]:
                    wT = emit_scores(1, p, hh)
                    if len(sc1_pend) >= 2:
                        a = sc1_pend.pop(0)
                        emit_z(1, *a)
                        if a[:2] == (2, 1):
                            den_stage_a(1)
                    sc1_pend.append((p, hh, wT))
                for a in sc1_pend:
                    emit_z(1, *a)
                    if a[:2] == (2, 1):
                        den_stage_a(1)
                normalize_cc(1)

                ff_half(0, rs0)
                ff_half(1, rs1)
                if DBG:
                    nc.sync.dma_start(dq[:], qT[:, 0, :])
                    nc.sync.dma_start(dv[:], vA[:, 0, :, :].rearrange("p h e -> p (h e)"))
                    nc.sync.dma_start(dnum[:], numT[:, 0, :])
                    nc.sync.dma_start(dden[:], denT[:HL, :])
                    nc.sync.dma_start(dzs[:], zsum[:])
                    nc.sync.dma_start(drs[:], rs0[:].bitcast(FP32))
                    nc.sync.dma_start(drp[:], recipT[:].bitcast(FP32))
                    nc.sync.dma_start(dsg[:], stageD[:])
                    nc.sync.dma_start(dtm[:], tmps[0][:].bitcast(FP32))

    nc.compile()
    return nc


_NC = None


def _get_nc():
    global _NC
    if _NC is None:
        _NC = build_nc()
    return _NC


def make_in_maps(x, Wq, bq, Wk, bk, Wv, bv, Wf, bf):
    x, Wq, bq, Wk, bk, Wv, bv, Wf, bf = (
        np.asarray(a, dtype=np.float32)
        for a in (x, Wq, bq, Wk, bk, Wv, bv, Wf, bf))

    r = np.arange(128)
    tri = (r[:, None] <= r[None, :]).astype(BF)          # key r allowed for query j
    sel = np.zeros((8, NPAIR, 128), np.float32)
    for p in range(NPAIR):
        for j in range(128):
            sel[2 * p + j // 64, p, j] = 1.0
    wfa = np.concatenate([Wf, bf.reshape(1, D)], axis=0).astype(BF)
    fold = (np.arange(128)[:, None] % 64 == np.arange(DH)[None, :]).astype(np.float32)

    in_maps = []
    for c in range(8):
        b, g = c // 2, c % 2
        hs = slice(g * HL, (g + 1) * HL)
        bqk_l = np.empty((128, 2 * NPAIR), np.float32)
        for p in range(NPAIR):
            bqk_l[:, p] = bq[g * HL + 2 * p: g * HL + 2 * p + 2].reshape(128)
            bqk_l[:, NPAIR + p] = bk[g * HL + 2 * p: g * HL + 2 * p + 2].reshape(128)
        in_maps.append({
            "xT": np.ascontiguousarray(x[b].T).astype(BF),
            "wq": np.ascontiguousarray(
                Wq[hs].transpose(1, 0, 2).reshape(D, HL * DH)).astype(BF),
            "wk": np.ascontiguousarray(
                Wk[hs].transpose(1, 0, 2).reshape(D, HL * DH)).astype(BF),
            "wv": np.ascontiguousarray(
                Wv[hs].transpose(1, 0, 2).reshape(D, HL * DH)).astype(BF),
            "bqk": bqk_l,
            "bvt": np.ascontiguousarray(
                np.broadcast_to(bv[hs].reshape(1, HL * DH), (128, HL * DH))),
            "tri01": tri,
            "onesp": np.ones((128, 8, HL), BF),
            "selp": sel,
            "foldp": fold,
            "wfa": wfa,
            "onesr": np.ones((1, 512), BF),
        })
    return in_maps


def run(in_maps, trace=False, **kw):
    nc = _get_nc()
    return run_bass_kernel_spmd(nc, in_maps, list(range(8)), trace=trace, **kw)


def assemble(results):
    """results: list of 8 per-core dicts -> full [B, S, D] output."""
    out = np.empty((B, S, D), np.float32)
    for b in range(B):
        out[b] = results[2 * b]["out"]
    return out


def bench(in_maps, iters=30, warmup=3):
    """Build the sharded PJRT executable once, run `iters` back-to-back
    executions with device-resident inputs, return (results, per_iter_ns).
    """
    import time

    import jax
    from jax.experimental.shard_map import shard_map
    from jax.sharding import Mesh, PartitionSpec

    from concourse import bass2jax, mybir as _mybir

    nc = _get_nc()
    bass2jax.install_neuronx_cc_hook()
    partition_name = nc.partition_id_tensor.name if nc.partition_id_tensor else None

    in_names, out_names, out_avals, zero_outs = [], [], [], []
    for alloc in nc.m.functions[0].allocations:
        if not isinstance(alloc, _mybir.MemoryLocationSet):
            continue
        name = alloc.memorylocations[0].name
        if alloc.kind == "ExternalInput":
            if name != partition_name:
                in_names.append(name)
        elif alloc.kind == "ExternalOutput":
            out_names.append(name)
            shape = tuple(alloc.tensor_shape)
            dtype = _mybir.dt.np(alloc.dtype)
            out_avals.append(jax.core.ShapedArray(shape, dtype))
            zero_outs.append(np.zeros(shape, dtype))
    n_params = len(in_names)

    all_in_names = list(in_names) + list(out_names)
    if partition_name is not None:
        all_in_names.append(partition_name)

    def _body2(*args):
        operands = list(args)
        if partition_name is not None:
            operands.append(bass2jax.partition_id_tensor())
        outs = bass2jax._bass_exec_p.bind(
            *operands,
            out_avals=tuple(out_avals),
            in_names=tuple(all_in_names),
            out_names=tuple(out_names),
            lowering_input_output_aliases=(),
            sim_require_finite=True,
            sim_require_nnan=True,
            nc=nc,
        )
        return tuple(outs)

    n_cores = 8
    devices = jax.devices()[:n_cores]
    mesh = Mesh(np.asarray(devices), ("core",))
    n_outs = len(out_names)
    sharded = jax.jit(
        shard_map(_body2, mesh=mesh,
                  in_specs=(PartitionSpec("core"),) * (n_params + n_outs),
                  out_specs=(PartitionSpec("core"),) * n_outs,
                  check_rep=False),
        keep_unused=True,
    )
    per_core = [[np.asarray(m[name]) for name in in_names] for m in in_maps]
    concat_in = [np.concatenate([per_core[c][i] for c in range(n_cores)], axis=0)
                 for i in range(n_params)]
    concat_zeros = [np.zeros((n_cores * z.shape[0], *z.shape[1:]), z.dtype)
                    for z in zero_outs]
    dev_in = [jax.device_put(a) for a in concat_in]
    dev_zero = [jax.device_put(a) for a in concat_zeros]

    out_arrs = jax.block_until_ready(sharded(*dev_in, *dev_zero))
    for _ in range(warmup - 1):
        out_arrs = jax.block_until_ready(sharded(*dev_in, *dev_zero))
    t0 = time.perf_counter()
    for _ in range(iters):
        out_arrs = sharded(*dev_in, *dev_zero)
    jax.block_until_ready(out_arrs)
    t1 = time.perf_counter()
    per_iter_ns = (t1 - t0) / iters * 1e9

    results = [
        {name: np.asarray(out_arrs[i]).reshape(n_cores, *out_avals[i].shape)[c]
         for i, name in enumerate(out_names)}
        for c in range(n_cores)
    ]
    return results, per_iter_ns


def kernel(x, Wq, bq, Wk, bk, Wv, bv, Wf, bf):
    in_maps = make_in_maps(x, Wq, bq, Wk, bk, Wv, bv, Wf, bf)
    res = run(in_maps)
    return assemble(res.results)


if __name__ == "__main__":
    nc = build_nc()
    print("build OK")


# revision 31
# speedup vs baseline: 1.4972x; 1.4972x over previous
"""Trainium2 Bass kernel for nn_AttentionLayer (dense transformer block with
summed heads), distributed over 8 NeuronCores.

Sharding: 4 batches x 2 head-groups (8 heads each), as the baseline — but
restructured for speed:
  - bf16 matmul datapath (fp32 PSUM), halving DMA + LDWEIGHTS traffic.
  - dc-outer projections with 8 PSUM accumulators so the first matmul only
    needs the first 128-row chunk of x/W (input DMA overlaps compute).
  - causal restriction: score/z matmuls only cover columns >= the block
    diagonal, so fully-masked regions are never computed and the softmax
    denominator is exact without additive -1e11 masking; only the diagonal
    128x128 triangle needs a 0/1 multiply.
  - softmax normalize: denominators for all 8 heads are batched into ONE
    vector reciprocal, broadcast across partitions with a one-hot matmul on
    the PE, and folded with a short tensor-tensor tree (replaces 16 x 3.3us
    reciprocals + 16 gpsimd broadcasts).
  - sequence-half-outer loop with one pairwise AllReduce per half: the
    first collective overlaps the second half's attention, and the first
    half's ff Dense runs during the second half too. Both cores of a pair
    compute the full [S, D] output (identical after AllReduce), host takes
    one copy.
"""

import sys

sys.path.insert(0, "/opt/trn_rl_repo")

import numpy as np
import ml_dtypes

import concourse.bass as bass
import concourse.bacc as bacc
import concourse.mybir as mybir
import concourse.tile as tile
from concourse.bass_utils import run_bass_kernel_spmd

B, S, D, H, DH = 4, 1024, 1024, 16, 64
HL, NPAIR = 8, 4          # heads / head-pairs per core
FP32 = mybir.dt.float32
F32R = mybir.dt.float32r
BF16 = mybir.dt.bfloat16
AF = mybir.ActivationFunctionType
ALU = mybir.AluOpType
RG = [[0, 1], [2, 3], [4, 5], [6, 7]]
BF = ml_dtypes.bfloat16


def _r(ap):
    return ap.bitcast(F32R)


def build_nc():
    nc = bacc.Bacc("TRN2", target_bir_lowering=False, num_devices=8)

    xT = nc.declare_dram_parameter("xT", [D, S], BF16, isOutput=False)
    wq = nc.declare_dram_parameter("wq", [D, HL * DH], BF16, isOutput=False)
    wk = nc.declare_dram_parameter("wk", [D, HL * DH], BF16, isOutput=False)
    wv = nc.declare_dram_parameter("wv", [D, HL * DH], BF16, isOutput=False)
    bqk = nc.declare_dram_parameter("bqk", [128, 2 * NPAIR], FP32, isOutput=False)
    bvt = nc.declare_dram_parameter("bvt", [128, HL * DH], FP32, isOutput=False)
    tri01 = nc.declare_dram_parameter("tri01", [128, 128], BF16, isOutput=False)
    onesp = nc.declare_dram_parameter("onesp", [128, 8, HL], BF16, isOutput=False)
    selp = nc.declare_dram_parameter("selp", [8, NPAIR, 128], FP32, isOutput=False)
    foldp = nc.declare_dram_parameter("foldp", [128, DH], FP32, isOutput=False)
    wfa = nc.declare_dram_parameter("wfa", [DH + 1, D], BF16, isOutput=False)
    onesr = nc.declare_dram_parameter("onesr", [1, 512], BF16, isOutput=False)
    out_ext = nc.declare_dram_parameter("out", [S, D], FP32, isOutput=True)
    import os
    DBG = os.environ.get("KDBG") == "1"
    if DBG:
        dq = nc.declare_dram_parameter("dq", [128, S], BF16, isOutput=True)
        dv = nc.declare_dram_parameter("dv", [128, HL * (DH + 1)], BF16, isOutput=True)
        dnum = nc.declare_dram_parameter("dnum", [128, S], BF16, isOutput=True)
        dden = nc.declare_dram_parameter("dden", [HL, S], FP32, isOutput=True)
        dzs = nc.declare_dram_parameter("dzs", [DH, S], FP32, isOutput=True)
        drs = nc.declare_dram_parameter("drs", [DH + 1, 512], FP32, isOutput=True)
        drp = nc.declare_dram_parameter("drp", [128, S], FP32, isOutput=True)
        dsg = nc.declare_dram_parameter("dsg", [1, HL, S], FP32, isOutput=True)
        dtm = nc.declare_dram_parameter("dtm", [128, 512], FP32, isOutput=True)

    with tile.TileContext(nc) as tc:
        with (
            tc.tile_pool(name="const", bufs=1) as constp,
            tc.tile_pool(name="qkv", bufs=1) as qkvp,
            tc.tile_pool(name="dram", bufs=1, space="DRAM") as dramp,
            tc.tile_pool(name="w0pool", bufs=9) as w0pool,
            tc.tile_pool(name="outp", bufs=3) as outp,
        ):
            # ---- persistent activations ----
            qT = qkvp.tile([128, NPAIR, S], BF16)   # [(2 heads)*64e, pair, q]
            kT = qkvp.tile([128, NPAIR, S], BF16)
            vA = qkvp.tile([128, 8, HL, DH + 1], BF16)  # [t_in, t_chunk, head, e|1]
            numT = qkvp.tile([128, NPAIR, S], BF16)  # normalized-pending z numerators
            denT = qkvp.tile([128, S], FP32)
            stageD = qkvp.tile([1, HL, S], FP32)
            recipT = qkvp.tile([128, S], F32R)
            zsum = qkvp.tile([DH, S], BF16)
            rs0 = qkvp.tile([DH + 1, 512], BF16)
            rs1 = qkvp.tile([DH + 1, 512], BF16)

            # ---- constants (emitted early; small) ----
            bqk_sb = constp.tile([128, 2 * NPAIR], FP32)
            bvt_sb = constp.tile([128, HL * DH], FP32)
            tri_sb = constp.tile([128, 128], BF16)
            sel_sb = constp.tile([8, NPAIR, 128], F32R)
            fold_sb = constp.tile([128, DH], F32R)
            wfa_sb = constp.tile([DH + 1, D], BF16)

            # ---- phase 1: projections (dc-outer, 8 PSUM accumulators) ----
            with (
                tc.tile_pool(name="xp", bufs=8) as xp,
                tc.tile_pool(name="wtp", bufs=3) as wtp,
                tc.tile_pool(name="projp", bufs=8, space="PSUM") as projp,
            ):
                xts = []
                wq_sb = wtp.tile([128, 8, HL * DH], BF16, tag="w")
                wk_sb = wtp.tile([128, 8, HL * DH], BF16, tag="w")
                wv_sb = wtp.tile([128, 8, HL * DH], BF16, tag="w")
                for dc in range(8):
                    t = xp.tile([128, S], BF16, tag="xT")
                    nc.sync.dma_start(t[:], xT[dc * 128:(dc + 1) * 128, :])
                    xts.append(t)
                    nc.scalar.dma_start(wq_sb[:, dc, :], wq[dc * 128:(dc + 1) * 128, :])
                for dc in range(8):
                    nc.scalar.dma_start(wk_sb[:, dc, :], wk[dc * 128:(dc + 1) * 128, :])
                for dc in range(8):
                    nc.scalar.dma_start(wv_sb[:, dc, :], wv[dc * 128:(dc + 1) * 128, :])
                nc.scalar.dma_start(bqk_sb[:], bqk[:])
                nc.scalar.dma_start(bvt_sb[:], bvt[:])
                nc.scalar.dma_start(tri_sb[:], tri01[:])
                nc.scalar.dma_start(vA[:, :, :, DH:DH + 1], onesp[:])
                nc.scalar.dma_start(sel_sb[:], selp[:].bitcast(F32R))
                nc.scalar.dma_start(fold_sb[:], foldp[:].bitcast(F32R))
                nc.scalar.dma_start(wfa_sb[:], wfa[:])

                # q then k: 8 live accumulators each, contraction (dc) outer
                for w_sb, dst, boff in ((wq_sb, qT, 0), (wk_sb, kT, NPAIR)):
                    pss = [projp.tile([128, 512], FP32, tag="pp", name=f"pp{i}") for i in range(8)]
                    for dc in range(8):
                        for i in range(8):
                            p, s2 = i // 2, i % 2
                            nc.tensor.matmul(
                                pss[i][:],
                                w_sb[:, dc, p * 128:(p + 1) * 128],
                                xts[dc][:, s2 * 512:(s2 + 1) * 512],
                                start=(dc == 0), stop=(dc == 7),
                            )
                    for i in range(8):
                        p, s2 = i // 2, i % 2
                        nc.vector.tensor_scalar_add(
                            dst[:, p, s2 * 512:(s2 + 1) * 512], pss[i][:],
                            bqk_sb[:, boff + p:boff + p + 1])

                # v: out rows = key chunks, two 4-slot passes with the first
                # sequence-half's score/exp stream interleaved between them
                def v_pass(lo4):
                    pss = [projp.tile([128, 512], FP32, tag="pp", name=f"pv{i}")
                           for i in range(4)]
                    for dc in range(8):
                        for i in range(4):
                            t_c = lo4 + i
                            nc.tensor.matmul(
                                pss[i][:],
                                xts[dc][:, t_c * 128:(t_c + 1) * 128],
                                wv_sb[:, dc, :],
                                start=(dc == 0), stop=(dc == 7),
                            )
                    for i in range(4):
                        t_c = lo4 + i
                        nc.vector.tensor_tensor(
                            vA[:, t_c, :, :DH],
                            pss[i][:].rearrange("p (h e) -> p h e", h=HL),
                            bvt_sb[:].rearrange("p (h e) -> p h e", h=HL),
                            ALU.add,
                        )

                def sc0_scores(p, hh):
                    rows = slice(hh * 64, hh * 64 + 64)
                    wT = w0pool.tile([128, 4, 512], BF16, tag="wT0", name="wT0")
                    for t_c in range(4):
                        lo = 128 * t_c
                        ps = projp.tile([128, 512], FP32, tag="pp", name="ps0")
                        nc.tensor.matmul(
                            ps[:, lo:],
                            kT[rows, p, t_c * 128:(t_c + 1) * 128],
                            qT[rows, p, lo:512],
                            start=True, stop=True,
                        )
                        nc.scalar.activation(
                            wT[:, t_c, lo:], ps[:, lo:], AF.Exp, scale=0.125)
                        nc.vector.tensor_tensor(
                            wT[:, t_c, lo:lo + 128],
                            wT[:, t_c, lo:lo + 128],
                            tri_sb[:], ALU.mult)
                    return wT

                w0s = []
                v_pass(0)
                w0s.append(sc0_scores(0, 0))
                w0s.append(sc0_scores(0, 1))
                v_pass(4)
                for p in range(NPAIR):
                    for hh in range(2):
                        if p == 0:
                            continue
                        w0s.append(sc0_scores(p, hh))

            # ---- phases 2-4: attention / normalize / collectives / ff ----
            with (
                tc.tile_pool(name="spsum", bufs=3, space="PSUM") as spsum,
                tc.tile_pool(name="zpsum", bufs=3, space="PSUM") as zpsum,
                tc.tile_pool(name="bcpsum", bufs=2, space="PSUM") as bcpsum,
                tc.tile_pool(name="wpool", bufs=4) as wpool,
                tc.tile_pool(name="smallp", bufs=3) as smallp,
            ):
                zouts = []
                nc.vector.memset(denT[:], 1.0)

                def ff_half(half, rs_t):
                    # rs_t rows 0..63 = allreduced z, row 64 = 1 (bias row of wfa)
                    nc.sync.dma_start(rs_t[:DH, :], zouts[half][:])
                    nc.sync.dma_start(rs_t[DH:DH + 1, :], onesr[:])
                    for j in range(4):
                        for dcol in range(2):
                            dsl = slice(dcol * 512, (dcol + 1) * 512)
                            fps = zpsum.tile([128, 512], FP32, tag="zp")
                            nc.tensor.matmul(
                                fps[:],
                                rs_t[:, j * 128:(j + 1) * 128],
                                wfa_sb[:, dsl],
                                start=True, stop=True,
                            )
                            og = outp.tile([128, 512], FP32, tag="og")
                            nc.scalar.activation(og[:], fps[:], AF.Gelu)
                            eng = nc.sync if (j + dcol) % 2 == 0 else nc.scalar
                            eng.dma_start(
                                out_ext[half * 512 + j * 128:half * 512 + (j + 1) * 128, dsl],
                                og[:])

                def emit_scores(sc, p, hh):
                    C = 4 * sc + 4
                    rows = slice(hh * 64, hh * 64 + 64)
                    wT = wpool.tile([128, 8, 512], BF16, tag="wT", name="wT")
                    for t_c in range(C):
                        lo = 128 * (t_c - 4 * sc) if t_c >= 4 * sc else 0
                        ps = spsum.tile([128, 512], FP32, tag="sp", name="ps")
                        nc.tensor.matmul(
                            ps[:, lo:],
                            kT[rows, p, t_c * 128:(t_c + 1) * 128],
                            qT[rows, p, sc * 512 + lo:(sc + 1) * 512],
                            start=True, stop=True,
                        )
                        nc.scalar.activation(
                            wT[:, t_c, lo:], ps[:, lo:], AF.Exp, scale=0.125)
                        if t_c >= 4 * sc:
                            nc.vector.tensor_tensor(
                                wT[:, t_c, lo:lo + 128],
                                wT[:, t_c, lo:lo + 128],
                                tri_sb[:], ALU.mult)
                    return wT

                def emit_z(sc, p, hh, wT):
                    C = 4 * sc + 4
                    ssl = slice(sc * 512, (sc + 1) * 512)
                    rows = slice(hh * 64, hh * 64 + 64)
                    h_loc = 2 * p + hh
                    zaug = zpsum.tile([128, 512], FP32, tag="zp", name="zaug")
                    for t_c in range(C):
                        lo = 128 * (t_c - 4 * sc) if t_c >= 4 * sc else 0
                        nc.tensor.matmul(
                            zaug[:DH + 1, lo:],
                            vA[:, t_c, h_loc, :],
                            wT[:, t_c, lo:],
                            start=(t_c == 0), stop=(t_c == C - 1),
                            skip_group_check=True,
                        )
                    nc.vector.tensor_copy(
                        stageD[0:1, h_loc, ssl], zaug[DH:DH + 1, :])
                    nc.vector.tensor_copy(
                        numT[rows, p, ssl], zaug[:DH, :])

                def den_stage_a(sc):
                    ssl = slice(sc * 512, (sc + 1) * 512)
                    t = dramp.tile([6, 512], FP32, name=f"ddena{sc}")
                    nc.sync.dma_start(t[:], stageD[0:1, 0:6, ssl])
                    nc.sync.dma_start(denT[0:6, ssl], t[:])

                def normalize_cc(sc):
                    ssl = slice(sc * 512, (sc + 1) * 512)
                    dden_b = dramp.tile([2, 512], FP32, name=f"ddenb{sc}")
                    nc.sync.dma_start(dden_b[:], stageD[0:1, 6:8, ssl])
                    nc.sync.dma_start(denT[6:8, ssl], dden_b[:])
                    with nc.allow_low_precision(reason="f32r is fp32 bits"):
                        nc.vector.reciprocal(recipT[:, ssl], denT[:, ssl])
                    tmps = []
                    for p in range(NPAIR):
                        bc = bcpsum.tile([128, 512], FP32, tag="bc")
                        nc.tensor.matmul(
                            bc[:], sel_sb[:, p, :], recipT[:HL, ssl],
                            start=True, stop=True,
                        )
                        tmp = smallp.tile([128, 512], F32R, tag=f"tmp{p}")
                        nc.vector.tensor_tensor(
                            tmp[:], numT[:, p, ssl], bc[:], ALU.mult)
                        tmps.append(tmp)
                    zps = bcpsum.tile([DH, 512], FP32, tag="bc", name="zps")
                    for p in range(NPAIR):
                        nc.tensor.matmul(
                            zps[:], fold_sb[:], tmps[p][:],
                            start=(p == 0), stop=(p == NPAIR - 1),
                        )
                    nc.vector.tensor_copy(zsum[:, ssl], zps[:])
                    zin = dramp.tile([DH, 512], BF16, name=f"zin{sc}")
                    nc.sync.dma_start(zin[:], zsum[:, ssl])
                    zout = dramp.tile([DH, 512], BF16, name=f"zout{sc}")
                    nc.gpsimd.collective_compute(
                        "AllReduce", ALU.add, replica_groups=RG,
                        ins=[zin[:].opt()], outs=[zout[:].opt()],
                    )
                    zouts.append(zout)

                GRP = [(p, hh) for p in range(NPAIR) for hh in range(2)]
                # first half: consume prefetched scores; interleave the second
                # half's first score groups so the exp stream never goes cold
                # across the first half's z-chains and CC0's issue window
                sc1_pend = []
                for gi, (p, hh) in enumerate(GRP):
                    emit_z(0, p, hh, w0s[gi])
                    if (p, hh) == (2, 1):
                        den_stage_a(0)
                    if gi in (1, 3):
                        g = GRP[len(sc1_pend)]
                        sc1_pend.append((g[0], g[1], emit_scores(1, *g)))
                normalize_cc(0)
                for (p, hh) in GRP[len(sc1_pend):]:
                    wT = emit_scores(1, p, hh)
                    if len(sc1_pend) >= 2:
                        a = sc1_pend.pop(0)
                        emit_z(1, *a)
                        if a[:2] == (2, 1):
                            den_stage_a(1)
                    sc1_pend.append((p, hh, wT))
                for a in sc1_pend:
                    emit_z(1, *a)
                    if a[:2] == (2, 1):
                        den_stage_a(1)
                normalize_cc(1)

                ff_half(0, rs0)
                ff_half(1, rs1)
                if DBG:
                    nc.sync.dma_start(dq[:], qT[:, 0, :])
                    nc.sync.dma_start(dv[:], vA[:, 0, :, :].rearrange("p h e -> p (h e)"))
                    nc.sync.dma_start(dnum[:], numT[:, 0, :])
                    nc.sync.dma_start(dden[:], denT[:HL, :])
                    nc.sync.dma_start(dzs[:], zsum[:])
                    nc.sync.dma_start(drs[:], rs0[:].bitcast(FP32))
                    nc.sync.dma_start(drp[:], recipT[:].bitcast(FP32))
                    nc.sync.dma_start(dsg[:], stageD[:])
                    nc.sync.dma_start(dtm[:], tmps[0][:].bitcast(FP32))

    nc.compile()
    return nc


_NC = None


def _get_nc():
    global _NC
    if _NC is None:
        _NC = build_nc()
    return _NC


def make_in_maps(x, Wq, bq, Wk, bk, Wv, bv, Wf, bf):
    x, Wq, bq, Wk, bk, Wv, bv, Wf, bf = (
        np.asarray(a, dtype=np.float32)
        for a in (x, Wq, bq, Wk, bk, Wv, bv, Wf, bf))

    r = np.arange(128)
    tri = (r[:, None] <= r[None, :]).astype(BF)          # key r allowed for query j
    sel = np.zeros((8, NPAIR, 128), np.float32)
    for p in range(NPAIR):
        for j in range(128):
            sel[2 * p + j // 64, p, j] = 1.0
    wfa = np.concatenate([Wf, bf.reshape(1, D)], axis=0).astype(BF)
    fold = (np.arange(128)[:, None] % 64 == np.arange(DH)[None, :]).astype(np.float32)

    in_maps = []
    for c in range(8):
        b, g = c // 2, c % 2
        hs = slice(g * HL, (g + 1) * HL)
        bqk_l = np.empty((128, 2 * NPAIR), np.float32)
        for p in range(NPAIR):
            bqk_l[:, p] = bq[g * HL + 2 * p: g * HL + 2 * p + 2].reshape(128)
            bqk_l[:, NPAIR + p] = bk[g * HL + 2 * p: g * HL + 2 * p + 2].reshape(128)
        in_maps.append({
            "xT": np.ascontiguousarray(x[b].T).astype(BF),
            "wq": np.ascontiguousarray(
                Wq[hs].transpose(1, 0, 2).reshape(D, HL * DH)).astype(BF),
            "wk": np.ascontiguousarray(
                Wk[hs].transpose(1, 0, 2).reshape(D, HL * DH)).astype(BF),
            "wv": np.ascontiguousarray(
                Wv[hs].transpose(1, 0, 2).reshape(D, HL * DH)).astype(BF),
            "bqk": bqk_l,
            "bvt": np.ascontiguousarray(
                np.broadcast_to(bv[hs].reshape(1, HL * DH), (128, HL * DH))),
            "tri01": tri,
            "onesp": np.ones((128, 8, HL), BF),
            "selp": sel,
            "foldp": fold,
            "wfa": wfa,
            "onesr": np.ones((1, 512), BF),
        })
    return in_maps


def run(in_maps, trace=False, **kw):
    nc = _get_nc()
    return run_bass_kernel_spmd(nc, in_maps, list(range(8)), trace=trace, **kw)


def assemble(results):
    """results: list of 8 per-core dicts -> full [B, S, D] output."""
    out = np.empty((B, S, D), np.float32)
    for b in range(B):
        out[b] = results[2 * b]["out"]
    return out


def bench(in_maps, iters=30, warmup=3):
    """Build the sharded PJRT executable once, run `iters` back-to-back
    executions with device-resident inputs, return (results, per_iter_ns).
    """
    import time

    import jax
    from jax.experimental.shard_map import shard_map
    from jax.sharding import Mesh, PartitionSpec

    from concourse import bass2jax, mybir as _mybir

    nc = _get_nc()
    bass2jax.install_neuronx_cc_hook()
    partition_name = nc.partition_id_tensor.name if nc.partition_id_tensor else None

    in_names, out_names, out_avals, zero_outs = [], [], [], []
    for alloc in nc.m.functions[0].allocations:
        if not isinstance(alloc, _mybir.MemoryLocationSet):
            continue
        name = alloc.memorylocations[0].name
        if alloc.kind == "ExternalInput":
            if name != partition_name:
                in_names.append(name)
        elif alloc.kind == "ExternalOutput":
            out_names.append(name)
            shape = tuple(alloc.tensor_shape)
            dtype = _mybir.dt.np(alloc.dtype)
            out_avals.append(jax.core.ShapedArray(shape, dtype))
            zero_outs.append(np.zeros(shape, dtype))
    n_params = len(in_names)

    all_in_names = list(in_names) + list(out_names)
    if partition_name is not None:
        all_in_names.append(partition_name)

    def _body2(*args):
        operands = list(args)
        if partition_name is not None:
            operands.append(bass2jax.partition_id_tensor())
        outs = bass2jax._bass_exec_p.bind(
            *operands,
            out_avals=tuple(out_avals),
            in_names=tuple(all_in_names),
            out_names=tuple(out_names),
            lowering_input_output_aliases=(),
            sim_require_finite=True,
            sim_require_nnan=True,
            nc=nc,
        )
        return tuple(outs)

    n_cores = 8
    devices = jax.devices()[:n_cores]
    mesh = Mesh(np.asarray(devices), ("core",))
    n_outs = len(out_names)
    sharded = jax.jit(
        shard_map(_body2, mesh=mesh,
                  in_specs=(PartitionSpec("core"),) * (n_params + n_outs),
                  out_specs=(PartitionSpec("core"),) * n_outs,
                  check_rep=False),
        keep_unused=True,
    )
    per_core = [[np.asarray(m[name]) for name in in_names] for m in in_maps]
    concat_in = [np.concatenate([per_core[c][i] for c in range(n_cores)], axis=0)
                 for i in range(n_params)]
    concat_zeros = [np.zeros((n_cores * z.shape[0], *z.shape[1:]), z.dtype)
                    for z in zero_outs]
    dev_in = [jax.device_put(a) for a in concat_in]
    dev_zero = [jax.device_put(a) for a in concat_zeros]

    out_arrs = jax.block_until_ready(sharded(*dev_in, *dev_zero))
    for _ in range(warmup - 1):
        out_arrs = jax.block_until_ready(sharded(*dev_in, *dev_zero))
    t0 = time.perf_counter()
    for _ in range(iters):
        out_arrs = sharded(*dev_in, *dev_zero)
    jax.block_until_ready(out_arrs)
    t1 = time.perf_counter()
    per_iter_ns = (t1 - t0) / iters * 1e9

    results = [
        {name: np.asarray(out_arrs[i]).reshape(n_cores, *out_avals[i].shape)[c]
         for i, name in enumerate(out_names)}
        for c in range(n_cores)
    ]
    return results, per_iter_ns


def kernel(x, Wq, bq, Wk, bk, Wv, bv, Wf, bf):
    in_maps = make_in_maps(x, Wq, bq, Wk, bk, Wv, bv, Wf, bf)
    res = run(in_maps)
    return assemble(res.results)


if __name__ == "__main__":
    nc = build_nc()
    print("build OK")


# revision 33
# speedup vs baseline: 1.5739x; 1.0513x over previous
"""Trainium2 Bass kernel for nn_AttentionLayer (dense transformer block with
summed heads), distributed over 8 NeuronCores.

Sharding: 4 batches x 2 head-groups (8 heads each), as the baseline — but
restructured for speed:
  - bf16 matmul datapath (fp32 PSUM), halving DMA + LDWEIGHTS traffic.
  - dc-outer projections with 8 PSUM accumulators so the first matmul only
    needs the first 128-row chunk of x/W (input DMA overlaps compute).
  - causal restriction: score/z matmuls only cover columns >= the block
    diagonal, so fully-masked regions are never computed and the softmax
    denominator is exact without additive -1e11 masking; only the diagonal
    128x128 triangle needs a 0/1 multiply.
  - softmax normalize: denominators for all 8 heads are batched into ONE
    vector reciprocal, broadcast across partitions with a one-hot matmul on
    the PE, and folded with a short tensor-tensor tree (replaces 16 x 3.3us
    reciprocals + 16 gpsimd broadcasts).
  - sequence-half-outer loop with one pairwise AllReduce per half: the
    first collective overlaps the second half's attention, and the first
    half's ff Dense runs during the second half too. Both cores of a pair
    compute the full [S, D] output (identical after AllReduce), host takes
    one copy.
"""

import sys

sys.path.insert(0, "/opt/trn_rl_repo")

import numpy as np
import ml_dtypes

import concourse.bass as bass
import concourse.bacc as bacc
import concourse.mybir as mybir
import concourse.tile as tile
from concourse.bass_utils import run_bass_kernel_spmd

B, S, D, H, DH = 4, 1024, 1024, 16, 64
HL, NPAIR = 8, 4          # heads / head-pairs per core
FP32 = mybir.dt.float32
F32R = mybir.dt.float32r
BF16 = mybir.dt.bfloat16
AF = mybir.ActivationFunctionType
ALU = mybir.AluOpType
RG = [[0, 1], [2, 3], [4, 5], [6, 7]]
BF = ml_dtypes.bfloat16


def _r(ap):
    return ap.bitcast(F32R)


def build_nc():
    nc = bacc.Bacc("TRN2", target_bir_lowering=False, num_devices=8)

    xT = nc.declare_dram_parameter("xT", [D, S], BF16, isOutput=False)
    wq = nc.declare_dram_parameter("wq", [D, HL * DH], BF16, isOutput=False)
    wk = nc.declare_dram_parameter("wk", [D, HL * DH], BF16, isOutput=False)
    wv = nc.declare_dram_parameter("wv", [D, HL * DH], BF16, isOutput=False)
    bqk = nc.declare_dram_parameter("bqk", [128, 2 * NPAIR], FP32, isOutput=False)
    bvt = nc.declare_dram_parameter("bvt", [128, HL * DH], FP32, isOutput=False)
    tri01 = nc.declare_dram_parameter("tri01", [128, 128], BF16, isOutput=False)
    onesp = nc.declare_dram_parameter("onesp", [128, 8, HL], BF16, isOutput=False)
    selp = nc.declare_dram_parameter("selp", [8, NPAIR, 128], FP32, isOutput=False)
    foldp = nc.declare_dram_parameter("foldp", [128, DH], FP32, isOutput=False)
    wfa = nc.declare_dram_parameter("wfa", [DH + 1, D], BF16, isOutput=False)
    onesr = nc.declare_dram_parameter("onesr", [1, 512], BF16, isOutput=False)
    out_ext = nc.declare_dram_parameter("out", [S, D], FP32, isOutput=True)
    import os
    DBG = os.environ.get("KDBG") == "1"
    if DBG:
        dq = nc.declare_dram_parameter("dq", [128, S], BF16, isOutput=True)
        dv = nc.declare_dram_parameter("dv", [128, HL * (DH + 1)], BF16, isOutput=True)
        dnum = nc.declare_dram_parameter("dnum", [128, S], BF16, isOutput=True)
        dden = nc.declare_dram_parameter("dden", [HL, S], FP32, isOutput=True)
        dzs = nc.declare_dram_parameter("dzs", [DH, S], FP32, isOutput=True)
        drs = nc.declare_dram_parameter("drs", [DH + 1, 512], FP32, isOutput=True)
        dsg = nc.declare_dram_parameter("dsg", [1, HL, S], FP32, isOutput=True)
        dtm = nc.declare_dram_parameter("dtm", [128, 512], FP32, isOutput=True)

    with tile.TileContext(nc) as tc:
        with (
            tc.tile_pool(name="const", bufs=1) as constp,
            tc.tile_pool(name="qkv", bufs=1) as qkvp,
            tc.tile_pool(name="dram", bufs=1, space="DRAM") as dramp,
            tc.tile_pool(name="w0pool", bufs=9) as w0pool,
            tc.tile_pool(name="outp", bufs=3) as outp,
        ):
            # ---- persistent activations ----
            qT = qkvp.tile([128, NPAIR, S], BF16)   # [(2 heads)*64e, pair, q]
            kT = qkvp.tile([128, NPAIR, S], BF16)
            vA = qkvp.tile([128, 8, HL, DH + 1], BF16)  # [t_in, t_chunk, head, e|1]
            numT = qkvp.tile([128, NPAIR, S], BF16)  # normalized-pending z numerators
            denT = qkvp.tile([128, S], FP32)
            recipT = qkvp.tile([128, S], F32R)
            stageD = qkvp.tile([1, HL, S], FP32)
            zsum = qkvp.tile([DH, S], BF16)
            rs0 = qkvp.tile([DH + 1, 512], BF16)
            rs1 = qkvp.tile([DH + 1, 512], BF16)

            # ---- constants (emitted early; small) ----
            bqk_sb = constp.tile([128, 2 * NPAIR], FP32)
            bvt_sb = constp.tile([128, HL * DH], FP32)
            tri_sb = constp.tile([128, 128], BF16)
            sel_sb = constp.tile([8, NPAIR, 128], F32R)
            fold_sb = constp.tile([128, DH], F32R)
            wfa_sb = constp.tile([DH + 1, D], BF16)

            # ---- phase 1: projections (dc-outer, 8 PSUM accumulators) ----
            with (
                tc.tile_pool(name="xp", bufs=8) as xp,
                tc.tile_pool(name="wtp", bufs=3) as wtp,
                tc.tile_pool(name="projp", bufs=8, space="PSUM") as projp,
            ):
                xts = []
                wq_sb = wtp.tile([128, 8, HL * DH], BF16, tag="w")
                wk_sb = wtp.tile([128, 8, HL * DH], BF16, tag="w")
                wv_sb = wtp.tile([128, 8, HL * DH], BF16, tag="w")
                for dc in range(8):
                    t = xp.tile([128, S], BF16, tag="xT")
                    nc.sync.dma_start(t[:], xT[dc * 128:(dc + 1) * 128, :])
                    xts.append(t)
                    if dc % 2 == 0:
                        nc.scalar.dma_start(
                            wq_sb[:, dc:dc + 2, :],
                            wq[dc * 128:(dc + 2) * 128, :].rearrange(
                                "(two p) c -> p two c", p=128))
                for dc in range(0, 8, 2):
                    nc.scalar.dma_start(
                        wk_sb[:, dc:dc + 2, :],
                        wk[dc * 128:(dc + 2) * 128, :].rearrange(
                            "(two p) c -> p two c", p=128))
                for dc in range(0, 8, 2):
                    nc.scalar.dma_start(
                        wv_sb[:, dc:dc + 2, :],
                        wv[dc * 128:(dc + 2) * 128, :].rearrange(
                            "(two p) c -> p two c", p=128))
                nc.scalar.dma_start(bqk_sb[:], bqk[:])
                nc.scalar.dma_start(bvt_sb[:], bvt[:])
                nc.scalar.dma_start(tri_sb[:], tri01[:])
                nc.scalar.dma_start(vA[:, :, :, DH:DH + 1], onesp[:])
                nc.scalar.dma_start(sel_sb[:], selp[:].bitcast(F32R))
                nc.scalar.dma_start(fold_sb[:], foldp[:].bitcast(F32R))
                nc.scalar.dma_start(wfa_sb[:], wfa[:])

                # q then k: 8 live accumulators each, contraction (dc) outer
                for w_sb, dst, boff in ((wq_sb, qT, 0), (wk_sb, kT, NPAIR)):
                    pss = [projp.tile([128, 512], FP32, tag="pp", name=f"pp{i}") for i in range(8)]
                    for dc in range(8):
                        for i in range(8):
                            p, s2 = i // 2, i % 2
                            nc.tensor.matmul(
                                pss[i][:],
                                w_sb[:, dc, p * 128:(p + 1) * 128],
                                xts[dc][:, s2 * 512:(s2 + 1) * 512],
                                start=(dc == 0), stop=(dc == 7),
                            )
                    for i in range(8):
                        p, s2 = i // 2, i % 2
                        nc.vector.tensor_scalar_add(
                            dst[:, p, s2 * 512:(s2 + 1) * 512], pss[i][:],
                            bqk_sb[:, boff + p:boff + p + 1])

                # v: out rows = key chunks, two 4-slot passes with the first
                # sequence-half's score/exp stream interleaved between them
                def v_pass(lo4):
                    pss = [projp.tile([128, 512], FP32, tag="pp", name=f"pv{i}")
                           for i in range(4)]
                    for dc in range(8):
                        for i in range(4):
                            t_c = lo4 + i
                            nc.tensor.matmul(
                                pss[i][:],
                                xts[dc][:, t_c * 128:(t_c + 1) * 128],
                                wv_sb[:, dc, :],
                                start=(dc == 0), stop=(dc == 7),
                            )
                    for i in range(4):
                        t_c = lo4 + i
                        nc.vector.tensor_tensor(
                            vA[:, t_c, :, :DH],
                            pss[i][:].rearrange("p (h e) -> p h e", h=HL),
                            bvt_sb[:].rearrange("p (h e) -> p h e", h=HL),
                            ALU.add,
                        )

                def sc0_scores(p, hh):
                    rows = slice(hh * 64, hh * 64 + 64)
                    wT = w0pool.tile([128, 4, 512], BF16, tag="wT0", name="wT0")
                    for t_c in range(4):
                        lo = 128 * t_c
                        ps = projp.tile([128, 512], FP32, tag="pp", name="ps0")
                        nc.tensor.matmul(
                            ps[:, lo:],
                            kT[rows, p, t_c * 128:(t_c + 1) * 128],
                            qT[rows, p, lo:512],
                            start=True, stop=True,
                        )
                        nc.scalar.activation(
                            wT[:, t_c, lo:], ps[:, lo:], AF.Exp, scale=0.125)
                        nc.vector.tensor_tensor(
                            wT[:, t_c, lo:lo + 128],
                            wT[:, t_c, lo:lo + 128],
                            tri_sb[:], ALU.mult)
                    return wT

                w0s = []
                v_pass(0)
                w0s.append(sc0_scores(0, 0))
                w0s.append(sc0_scores(0, 1))
                v_pass(4)
                for p in range(NPAIR):
                    for hh in range(2):
                        if p == 0:
                            continue
                        w0s.append(sc0_scores(p, hh))

            # ---- phases 2-4: attention / normalize / collectives / ff ----
            with (
                tc.tile_pool(name="spsum", bufs=3, space="PSUM") as spsum,
                tc.tile_pool(name="zpsum", bufs=3, space="PSUM") as zpsum,
                tc.tile_pool(name="bcpsum", bufs=2, space="PSUM") as bcpsum,
                tc.tile_pool(name="wpool", bufs=4) as wpool,
                tc.tile_pool(name="smallp", bufs=3) as smallp,
            ):
                zouts = []
                nc.vector.memset(denT[:], 1.0)

                def ff_half(half, rs_t):
                    # rs_t rows 0..63 = allreduced z, row 64 = 1 (bias row of wfa)
                    nc.sync.dma_start(rs_t[:DH, :], zouts[half][:])
                    nc.sync.dma_start(rs_t[DH:DH + 1, :], onesr[:])
                    for j in range(4):
                        for dcol in range(2):
                            dsl = slice(dcol * 512, (dcol + 1) * 512)
                            fps = zpsum.tile([128, 512], FP32, tag="zp")
                            nc.tensor.matmul(
                                fps[:],
                                rs_t[:, j * 128:(j + 1) * 128],
                                wfa_sb[:, dsl],
                                start=True, stop=True,
                            )
                            og = outp.tile([128, 512], FP32, tag="og")
                            nc.scalar.activation(og[:], fps[:], AF.Gelu)
                            eng = nc.sync if (j + dcol) % 2 == 0 else nc.scalar
                            eng.dma_start(
                                out_ext[half * 512 + j * 128:half * 512 + (j + 1) * 128, dsl],
                                og[:])

                def emit_scores(sc, p, hh):
                    C = 4 * sc + 4
                    rows = slice(hh * 64, hh * 64 + 64)
                    wT = wpool.tile([128, 8, 512], BF16, tag="wT", name="wT")
                    for t_c in range(C):
                        lo = 128 * (t_c - 4 * sc) if t_c >= 4 * sc else 0
                        ps = spsum.tile([128, 512], FP32, tag="sp", name="ps")
                        nc.tensor.matmul(
                            ps[:, lo:],
                            kT[rows, p, t_c * 128:(t_c + 1) * 128],
                            qT[rows, p, sc * 512 + lo:(sc + 1) * 512],
                            start=True, stop=True,
                        )
                        nc.scalar.activation(
                            wT[:, t_c, lo:], ps[:, lo:], AF.Exp, scale=0.125)
                        if t_c >= 4 * sc:
                            nc.vector.tensor_tensor(
                                wT[:, t_c, lo:lo + 128],
                                wT[:, t_c, lo:lo + 128],
                                tri_sb[:], ALU.mult)
                    return wT

                def emit_z(sc, p, hh, wT):
                    C = 4 * sc + 4
                    ssl = slice(sc * 512, (sc + 1) * 512)
                    rows = slice(hh * 64, hh * 64 + 64)
                    h_loc = 2 * p + hh
                    zaug = zpsum.tile([128, 512], FP32, tag="zp", name="zaug")
                    for t_c in range(C):
                        lo = 128 * (t_c - 4 * sc) if t_c >= 4 * sc else 0
                        nc.tensor.matmul(
                            zaug[:DH + 1, lo:],
                            vA[:, t_c, h_loc, :],
                            wT[:, t_c, lo:],
                            start=(t_c == 0), stop=(t_c == C - 1),
                            skip_group_check=True,
                        )
                    nc.vector.tensor_copy(
                        stageD[0:1, h_loc, ssl], zaug[DH:DH + 1, :])
                    nc.vector.tensor_copy(
                        numT[rows, p, ssl], zaug[:DH, :])

                def den_stage_a(sc):
                    ssl = slice(sc * 512, (sc + 1) * 512)
                    t = dramp.tile([6, 512], FP32, name=f"ddena{sc}")
                    nc.sync.dma_start(t[:], stageD[0:1, 0:6, ssl])
                    nc.sync.dma_start(denT[0:6, ssl], t[:])

                def normalize_cc(sc):
                    ssl = slice(sc * 512, (sc + 1) * 512)
                    dden_b = dramp.tile([2, 512], FP32, name=f"ddenb{sc}")
                    nc.sync.dma_start(dden_b[:], stageD[0:1, 6:8, ssl])
                    nc.sync.dma_start(denT[6:8, ssl], dden_b[:])
                    with nc.allow_low_precision(reason="f32r is fp32 bits"):
                        nc.vector.reciprocal(recipT[:, ssl], denT[:, ssl])
                    tmps = []
                    for p in range(NPAIR):
                        bc = bcpsum.tile([128, 512], FP32, tag="bc")
                        nc.tensor.matmul(
                            bc[:], sel_sb[:, p, :], recipT[:HL, ssl],
                            start=True, stop=True,
                        )
                        tmp = smallp.tile([128, 512], F32R, tag=f"tmp{p}")
                        nc.vector.tensor_tensor(
                            tmp[:], numT[:, p, ssl], bc[:], ALU.mult)
                        tmps.append(tmp)
                    zps = bcpsum.tile([DH, 512], FP32, tag="bc", name="zps")
                    for p in range(NPAIR):
                        nc.tensor.matmul(
                            zps[:], fold_sb[:], tmps[p][:],
                            start=(p == 0), stop=(p == NPAIR - 1),
                        )
                    nc.vector.tensor_copy(zsum[:, ssl], zps[:])
                    zin = dramp.tile([DH, 512], BF16, name=f"zin{sc}")
                    nc.sync.dma_start(zin[:], zsum[:, ssl])
                    zout = dramp.tile([DH, 512], BF16, name=f"zout{sc}")
                    nc.gpsimd.collective_compute(
                        "AllReduce", ALU.add, replica_groups=RG,
                        ins=[zin[:].opt()], outs=[zout[:].opt()],
                    )
                    zouts.append(zout)

                GRP = [(p, hh) for p in range(NPAIR) for hh in range(2)]
                # first half: consume prefetched scores; interleave the second
                # half's first score groups so the exp stream never goes cold
                # across the first half's z-chains and CC0's issue window
                sc1_pend = []
                for gi, (p, hh) in enumerate(GRP):
                    emit_z(0, p, hh, w0s[gi])
                    if (p, hh) == (2, 1):
                        den_stage_a(0)
                    if gi in (1, 3):
                        g = GRP[len(sc1_pend)]
                        sc1_pend.append((g[0], g[1], emit_scores(1, *g)))
                normalize_cc(0)
                for (p, hh) in GRP[len(sc1_pend):]:
                    wT = emit_scores(1, p, hh)
                    if len(sc1_pend) >= 2:
                        a = sc1_pend.pop(0)
                        emit_z(1, *a)
                        if a[:2] == (2, 1):
                            den_stage_a(1)
                    sc1_pend.append((p, hh, wT))
                for a in sc1_pend:
                    emit_z(1, *a)
                    if a[:2] == (2, 1):
                        den_stage_a(1)
                normalize_cc(1)

                ff_half(0, rs0)
                ff_half(1, rs1)
                if DBG:
                    nc.sync.dma_start(dq[:], qT[:, 0, :])
                    nc.sync.dma_start(dv[:], vA[:, 0, :, :].rearrange("p h e -> p (h e)"))
                    nc.sync.dma_start(dnum[:], numT[:, 0, :])
                    nc.sync.dma_start(dden[:], denT[:HL, :])
                    nc.sync.dma_start(dzs[:], zsum[:])
                    nc.sync.dma_start(drs[:], rs0[:].bitcast(FP32))
                    nc.sync.dma_start(dsg[:], stageD[:])
                    nc.sync.dma_start(dtm[:], tmps[0][:].bitcast(FP32))

    nc.compile()
    return nc


_NC = None


def _get_nc():
    global _NC
    if _NC is None:
        _NC = build_nc()
    return _NC


def make_in_maps(x, Wq, bq, Wk, bk, Wv, bv, Wf, bf):
    x, Wq, bq, Wk, bk, Wv, bv, Wf, bf = (
        np.asarray(a, dtype=np.float32)
        for a in (x, Wq, bq, Wk, bk, Wv, bv, Wf, bf))

    r = np.arange(128)
    tri = (r[:, None] <= r[None, :]).astype(BF)          # key r allowed for query j
    sel = np.zeros((8, NPAIR, 128), np.float32)
    for p in range(NPAIR):
        for j in range(128):
            sel[2 * p + j // 64, p, j] = 1.0
    wfa = np.concatenate([Wf, bf.reshape(1, D)], axis=0).astype(BF)
    fold = (np.arange(128)[:, None] % 64 == np.arange(DH)[None, :]).astype(np.float32)

    in_maps = []
    for c in range(8):
        b, g = c // 2, c % 2
        hs = slice(g * HL, (g + 1) * HL)
        bqk_l = np.empty((128, 2 * NPAIR), np.float32)
        for p in range(NPAIR):
            bqk_l[:, p] = bq[g * HL + 2 * p: g * HL + 2 * p + 2].reshape(128)
            bqk_l[:, NPAIR + p] = bk[g * HL + 2 * p: g * HL + 2 * p + 2].reshape(128)
        in_maps.append({
            "xT": np.ascontiguousarray(x[b].T).astype(BF),
            "wq": np.ascontiguousarray(
                Wq[hs].transpose(1, 0, 2).reshape(D, HL * DH)).astype(BF),
            "wk": np.ascontiguousarray(
                Wk[hs].transpose(1, 0, 2).reshape(D, HL * DH)).astype(BF),
            "wv": np.ascontiguousarray(
                Wv[hs].transpose(1, 0, 2).reshape(D, HL * DH)).astype(BF),
            "bqk": bqk_l,
            "bvt": np.ascontiguousarray(
                np.broadcast_to(bv[hs].reshape(1, HL * DH), (128, HL * DH))),
            "tri01": tri,
            "onesp": np.ones((128, 8, HL), BF),
            "selp": sel,
            "foldp": fold,
            "wfa": wfa,
            "onesr": np.ones((1, 512), BF),
        })
    return in_maps


def run(in_maps, trace=False, **kw):
    nc = _get_nc()
    return run_bass_kernel_spmd(nc, in_maps, list(range(8)), trace=trace, **kw)


def assemble(results):
    """results: list of 8 per-core dicts -> full [B, S, D] output."""
    out = np.empty((B, S, D), np.float32)
    for b in range(B):
        out[b] = results[2 * b]["out"]
    return out


def bench(in_maps, iters=30, warmup=3):
    """Build the sharded PJRT executable once, run `iters` back-to-back
    executions with device-resident inputs, return (results, per_iter_ns).
    """
    import time

    import jax
    from jax.experimental.shard_map import shard_map
    from jax.sharding import Mesh, PartitionSpec

    from concourse import bass2jax, mybir as _mybir

    nc = _get_nc()
    bass2jax.install_neuronx_cc_hook()
    partition_name = nc.partition_id_tensor.name if nc.partition_id_tensor else None

    in_names, out_names, out_avals, zero_outs = [], [], [], []
    for alloc in nc.m.functions[0].allocations:
        if not isinstance(alloc, _mybir.MemoryLocationSet):
            continue
        name = alloc.memorylocations[0].name
        if alloc.kind == "ExternalInput":
            if name != partition_name:
                in_names.append(name)
        elif alloc.kind == "ExternalOutput":
            out_names.append(name)
            shape = tuple(alloc.tensor_shape)
            dtype = _mybir.dt.np(alloc.dtype)
            out_avals.append(jax.core.ShapedArray(shape, dtype))
            zero_outs.append(np.zeros(shape, dtype))
    n_params = len(in_names)

    all_in_names = list(in_names) + list(out_names)
    if partition_name is not None:
        all_in_names.append(partition_name)

    def _body2(*args):
        operands = list(args)
        if partition_name is not None:
            operands.append(bass2jax.partition_id_tensor())
        outs = bass2jax._bass_exec_p.bind(
            *operands,
            out_avals=tuple(out_avals),
            in_names=tuple(all_in_names),
            out_names=tuple(out_names),
            lowering_input_output_aliases=(),
            sim_require_finite=True,
            sim_require_nnan=True,
            nc=nc,
        )
        return tuple(outs)

    n_cores = 8
    devices = jax.devices()[:n_cores]
    mesh = Mesh(np.asarray(devices), ("core",))
    n_outs = len(out_names)
    sharded = jax.jit(
        shard_map(_body2, mesh=mesh,
                  in_specs=(PartitionSpec("core"),) * (n_params + n_outs),
                  out_specs=(PartitionSpec("core"),) * n_outs,
                  check_rep=False),
        keep_unused=True,
    )
    per_core = [[np.asarray(m[name]) for name in in_names] for m in in_maps]
    concat_in = [np.concatenate([per_core[c][i] for c in range(n_cores)], axis=0)
                 for i in range(n_params)]
    concat_zeros = [np.zeros((n_cores * z.shape[0], *z.shape[1:]), z.dtype)
                    for z in zero_outs]
    dev_in = [jax.device_put(a) for a in concat_in]
    dev_zero = [jax.device_put(a) for a in concat_zeros]

    out_arrs = jax.block_until_ready(sharded(*dev_in, *dev_zero))
    for _ in range(warmup - 1):
        out_arrs = jax.block_until_ready(sharded(*dev_in, *dev_zero))
    t0 = time.perf_counter()
    for _ in range(iters):
        out_arrs = sharded(*dev_in, *dev_zero)
    jax.block_until_ready(out_arrs)
    t1 = time.perf_counter()
    per_iter_ns = (t1 - t0) / iters * 1e9

    results = [
        {name: np.asarray(out_arrs[i]).reshape(n_cores, *out_avals[i].shape)[c]
         for i, name in enumerate(out_names)}
        for c in range(n_cores)
    ]
    return results, per_iter_ns


def kernel(x, Wq, bq, Wk, bk, Wv, bv, Wf, bf):
    in_maps = make_in_maps(x, Wq, bq, Wk, bk, Wv, bv, Wf, bf)
    res = run(in_maps)
    return assemble(res.results)


if __name__ == "__main__":
    nc = build_nc()
    print("build OK")


# revision 34
# speedup vs baseline: 1.5920x; 1.0115x over previous
"""Trainium2 Bass kernel for nn_AttentionLayer (dense transformer block with
summed heads), distributed over 8 NeuronCores.

Sharding: 4 batches x 2 head-groups (8 heads each), as the baseline — but
restructured for speed:
  - bf16 matmul datapath (fp32 PSUM), halving DMA + LDWEIGHTS traffic.
  - dc-outer projections with 8 PSUM accumulators so the first matmul only
    needs the first 128-row chunk of x/W (input DMA overlaps compute).
  - causal restriction: score/z matmuls only cover columns >= the block
    diagonal, so fully-masked regions are never computed and the softmax
    denominator is exact without additive -1e11 masking; only the diagonal
    128x128 triangle needs a 0/1 multiply.
  - softmax normalize: denominators for all 8 heads are batched into ONE
    vector reciprocal, broadcast across partitions with a one-hot matmul on
    the PE, and folded with a short tensor-tensor tree (replaces 16 x 3.3us
    reciprocals + 16 gpsimd broadcasts).
  - sequence-half-outer loop with one pairwise AllReduce per half: the
    first collective overlaps the second half's attention, and the first
    half's ff Dense runs during the second half too. Both cores of a pair
    compute the full [S, D] output (identical after AllReduce), host takes
    one copy.
"""

import sys

sys.path.insert(0, "/opt/trn_rl_repo")

import numpy as np
import ml_dtypes

import concourse.bass as bass
import concourse.bacc as bacc
import concourse.mybir as mybir
import concourse.tile as tile
from concourse.bass_utils import run_bass_kernel_spmd

B, S, D, H, DH = 4, 1024, 1024, 16, 64
HL, NPAIR = 8, 4          # heads / head-pairs per core
FP32 = mybir.dt.float32
F32R = mybir.dt.float32r
BF16 = mybir.dt.bfloat16
AF = mybir.ActivationFunctionType
ALU = mybir.AluOpType
RG = [[0, 1], [2, 3], [4, 5], [6, 7]]
BF = ml_dtypes.bfloat16


def _r(ap):
    return ap.bitcast(F32R)


def build_nc():
    nc = bacc.Bacc("TRN2", target_bir_lowering=False, num_devices=8)

    xT = nc.declare_dram_parameter("xT", [D, S], BF16, isOutput=False)
    wq = nc.declare_dram_parameter("wq", [D, HL * DH], BF16, isOutput=False)
    wk = nc.declare_dram_parameter("wk", [D, HL * DH], BF16, isOutput=False)
    wv = nc.declare_dram_parameter("wv", [D, HL * DH], BF16, isOutput=False)
    bqk = nc.declare_dram_parameter("bqk", [128, 2 * NPAIR], FP32, isOutput=False)
    bvt = nc.declare_dram_parameter("bvt", [128, HL * DH], FP32, isOutput=False)
    tri01 = nc.declare_dram_parameter("tri01", [128, 128], BF16, isOutput=False)
    onesp = nc.declare_dram_parameter("onesp", [128, 8, HL], BF16, isOutput=False)
    selp = nc.declare_dram_parameter("selp", [8, NPAIR, 128], FP32, isOutput=False)
    foldp = nc.declare_dram_parameter("foldp", [128, DH], FP32, isOutput=False)
    wfa = nc.declare_dram_parameter("wfa", [DH + 1, D], BF16, isOutput=False)
    onesr = nc.declare_dram_parameter("onesr", [1, 512], BF16, isOutput=False)
    out_ext = nc.declare_dram_parameter("out", [S, D], FP32, isOutput=True)
    import os
    DBG = os.environ.get("KDBG") == "1"
    if DBG:
        dq = nc.declare_dram_parameter("dq", [128, S], BF16, isOutput=True)
        dv = nc.declare_dram_parameter("dv", [128, HL * (DH + 1)], BF16, isOutput=True)
        dnum = nc.declare_dram_parameter("dnum", [128, S], BF16, isOutput=True)
        dden = nc.declare_dram_parameter("dden", [HL, S], FP32, isOutput=True)
        dzs = nc.declare_dram_parameter("dzs", [DH, S], FP32, isOutput=True)
        drs = nc.declare_dram_parameter("drs", [DH + 1, 512], FP32, isOutput=True)
        dsg = nc.declare_dram_parameter("dsg", [1, HL, S], FP32, isOutput=True)
        dtm = nc.declare_dram_parameter("dtm", [128, 512], FP32, isOutput=True)

    with tile.TileContext(nc) as tc:
        with (
            tc.tile_pool(name="const", bufs=1) as constp,
            tc.tile_pool(name="qkv", bufs=1) as qkvp,
            tc.tile_pool(name="dram", bufs=1, space="DRAM") as dramp,
            tc.tile_pool(name="w0pool", bufs=9) as w0pool,
            tc.tile_pool(name="outp", bufs=3) as outp,
        ):
            # ---- persistent activations ----
            qT = qkvp.tile([128, NPAIR, S], BF16)   # [(2 heads)*64e, pair, q]
            kT = qkvp.tile([128, NPAIR, S], BF16)
            vA = qkvp.tile([128, 8, HL, DH + 1], BF16)  # [t_in, t_chunk, head, e|1]
            numT = qkvp.tile([128, NPAIR, S], BF16)  # normalized-pending z numerators
            denT = qkvp.tile([128, S], FP32)
            recipT = qkvp.tile([128, S], F32R)
            stageD = qkvp.tile([1, HL, S], FP32)
            zsum = qkvp.tile([DH, S], BF16)
            rs0 = qkvp.tile([DH + 1, 512], BF16)
            rs1 = qkvp.tile([DH + 1, 512], BF16)

            # ---- constants (emitted early; small) ----
            bqk_sb = constp.tile([128, 2 * NPAIR], FP32)
            bvt_sb = constp.tile([128, HL * DH], FP32)
            tri_sb = constp.tile([128, 128], BF16)
            sel_sb = constp.tile([8, NPAIR, 128], F32R)
            fold_sb = constp.tile([128, DH], F32R)
            wfa_sb = constp.tile([DH + 1, D], BF16)

            # ---- phase 1: projections (dc-outer, 8 PSUM accumulators) ----
            with (
                tc.tile_pool(name="xp", bufs=8) as xp,
                tc.tile_pool(name="wtp", bufs=3) as wtp,
                tc.tile_pool(name="projp", bufs=8, space="PSUM") as projp,
            ):
                xts = []
                wq_sb = wtp.tile([128, 8, HL * DH], BF16, tag="w")
                wk_sb = wtp.tile([128, 8, HL * DH], BF16, tag="w")
                wv_sb = wtp.tile([128, 8, HL * DH], BF16, tag="w")
                xpair = qkvp.tile([128, 8, S], BF16)
                for dc in range(8):
                    xts.append(xpair[:, dc, :])
                for dc in range(8):
                    if dc % 2 == 0:
                        nc.sync.dma_start(
                            xpair[:, dc:dc + 2, :],
                            xT[dc * 128:(dc + 2) * 128, :].rearrange(
                                "(two p) c -> p two c", p=128))
                        nc.scalar.dma_start(
                            wq_sb[:, dc:dc + 2, :],
                            wq[dc * 128:(dc + 2) * 128, :].rearrange(
                                "(two p) c -> p two c", p=128))
                for dc in range(0, 8, 2):
                    nc.scalar.dma_start(
                        wk_sb[:, dc:dc + 2, :],
                        wk[dc * 128:(dc + 2) * 128, :].rearrange(
                            "(two p) c -> p two c", p=128))
                for dc in range(0, 8, 2):
                    nc.scalar.dma_start(
                        wv_sb[:, dc:dc + 2, :],
                        wv[dc * 128:(dc + 2) * 128, :].rearrange(
                            "(two p) c -> p two c", p=128))
                nc.scalar.dma_start(bqk_sb[:], bqk[:])
                nc.scalar.dma_start(bvt_sb[:], bvt[:])
                nc.scalar.dma_start(tri_sb[:], tri01[:])
                nc.scalar.dma_start(vA[:, :, :, DH:DH + 1], onesp[:])
                nc.scalar.dma_start(sel_sb[:], selp[:].bitcast(F32R))
                nc.scalar.dma_start(fold_sb[:], foldp[:].bitcast(F32R))
                nc.scalar.dma_start(wfa_sb[:], wfa[:])

                # q then k: 8 live accumulators each, contraction (dc) outer
                for w_sb, dst, boff in ((wq_sb, qT, 0), (wk_sb, kT, NPAIR)):
                    pss = [projp.tile([128, 512], FP32, tag="pp", name=f"pp{i}") for i in range(8)]
                    for dc in range(8):
                        for i in range(8):
                            p, s2 = i // 2, i % 2
                            nc.tensor.matmul(
                                pss[i][:],
                                w_sb[:, dc, p * 128:(p + 1) * 128],
                                xts[dc][:, s2 * 512:(s2 + 1) * 512],
                                start=(dc == 0), stop=(dc == 7),
                            )
                    for i in range(8):
                        p, s2 = i // 2, i % 2
                        nc.vector.tensor_scalar_add(
                            dst[:, p, s2 * 512:(s2 + 1) * 512], pss[i][:],
                            bqk_sb[:, boff + p:boff + p + 1])

                # v: out rows = key chunks, two 4-slot passes with the first
                # sequence-half's score/exp stream interleaved between them
                def v_pass(lo4):
                    pss = [projp.tile([128, 512], FP32, tag="pp", name=f"pv{i}")
                           for i in range(4)]
                    for dc in range(8):
                        for i in range(4):
                            t_c = lo4 + i
                            nc.tensor.matmul(
                                pss[i][:],
                                xts[dc][:, t_c * 128:(t_c + 1) * 128],
                                wv_sb[:, dc, :],
                                start=(dc == 0), stop=(dc == 7),
                            )
                    for i in range(4):
                        t_c = lo4 + i
                        nc.vector.tensor_tensor(
                            vA[:, t_c, :, :DH],
                            pss[i][:].rearrange("p (h e) -> p h e", h=HL),
                            bvt_sb[:].rearrange("p (h e) -> p h e", h=HL),
                            ALU.add,
                        )

                def sc0_scores(p, hh):
                    rows = slice(hh * 64, hh * 64 + 64)
                    wT = w0pool.tile([128, 4, 512], BF16, tag="wT0", name="wT0")
                    for t_c in range(4):
                        lo = 128 * t_c
                        ps = projp.tile([128, 512], FP32, tag="pp", name="ps0")
                        nc.tensor.matmul(
                            ps[:, lo:],
                            kT[rows, p, t_c * 128:(t_c + 1) * 128],
                            qT[rows, p, lo:512],
                            start=True, stop=True,
                        )
                        nc.scalar.activation(
                            wT[:, t_c, lo:], ps[:, lo:], AF.Exp, scale=0.125)
                        nc.vector.tensor_tensor(
                            wT[:, t_c, lo:lo + 128],
                            wT[:, t_c, lo:lo + 128],
                            tri_sb[:], ALU.mult)
                    return wT

                w0s = []
                v_pass(0)
                w0s.append(sc0_scores(0, 0))
                w0s.append(sc0_scores(0, 1))
                v_pass(4)
                for p in range(NPAIR):
                    for hh in range(2):
                        if p == 0:
                            continue
                        w0s.append(sc0_scores(p, hh))

            # ---- phases 2-4: attention / normalize / collectives / ff ----
            with (
                tc.tile_pool(name="spsum", bufs=3, space="PSUM") as spsum,
                tc.tile_pool(name="zpsum", bufs=3, space="PSUM") as zpsum,
                tc.tile_pool(name="bcpsum", bufs=2, space="PSUM") as bcpsum,
                tc.tile_pool(name="wpool", bufs=4) as wpool,
                tc.tile_pool(name="smallp", bufs=3) as smallp,
            ):
                zouts = []
                nc.vector.memset(denT[:], 1.0)

                def ff_half(half, rs_t):
                    # rs_t rows 0..63 = allreduced z, row 64 = 1 (bias row of wfa)
                    nc.sync.dma_start(rs_t[:DH, :], zouts[half][:])
                    nc.sync.dma_start(rs_t[DH:DH + 1, :], onesr[:])
                    for j in range(4):
                        for dcol in range(2):
                            dsl = slice(dcol * 512, (dcol + 1) * 512)
                            fps = zpsum.tile([128, 512], FP32, tag="zp")
                            nc.tensor.matmul(
                                fps[:],
                                rs_t[:, j * 128:(j + 1) * 128],
                                wfa_sb[:, dsl],
                                start=True, stop=True,
                            )
                            og = outp.tile([128, 512], FP32, tag="og")
                            nc.scalar.activation(og[:], fps[:], AF.Gelu)
                            eng = nc.sync if (j + dcol) % 2 == 0 else nc.scalar
                            eng.dma_start(
                                out_ext[half * 512 + j * 128:half * 512 + (j + 1) * 128, dsl],
                                og[:])

                def emit_scores(sc, p, hh):
                    C = 4 * sc + 4
                    rows = slice(hh * 64, hh * 64 + 64)
                    wT = wpool.tile([128, 8, 512], BF16, tag="wT", name="wT")
                    for t_c in range(C):
                        lo = 128 * (t_c - 4 * sc) if t_c >= 4 * sc else 0
                        ps = spsum.tile([128, 512], FP32, tag="sp", name="ps")
                        nc.tensor.matmul(
                            ps[:, lo:],
                            kT[rows, p, t_c * 128:(t_c + 1) * 128],
                            qT[rows, p, sc * 512 + lo:(sc + 1) * 512],
                            start=True, stop=True,
                        )
                        nc.scalar.activation(
                            wT[:, t_c, lo:], ps[:, lo:], AF.Exp, scale=0.125)
                        if t_c >= 4 * sc:
                            nc.vector.tensor_tensor(
                                wT[:, t_c, lo:lo + 128],
                                wT[:, t_c, lo:lo + 128],
                                tri_sb[:], ALU.mult)
                    return wT

                def emit_z(sc, p, hh, wT):
                    C = 4 * sc + 4
                    ssl = slice(sc * 512, (sc + 1) * 512)
                    rows = slice(hh * 64, hh * 64 + 64)
                    h_loc = 2 * p + hh
                    zaug = zpsum.tile([128, 512], FP32, tag="zp", name="zaug")
                    for t_c in range(C):
                        lo = 128 * (t_c - 4 * sc) if t_c >= 4 * sc else 0
                        nc.tensor.matmul(
                            zaug[:DH + 1, lo:],
                            vA[:, t_c, h_loc, :],
                            wT[:, t_c, lo:],
                            start=(t_c == 0), stop=(t_c == C - 1),
                            skip_group_check=True,
                        )
                    nc.vector.tensor_copy(
                        stageD[0:1, h_loc, ssl], zaug[DH:DH + 1, :])
                    nc.vector.tensor_copy(
                        numT[rows, p, ssl], zaug[:DH, :])

                def den_stage_a(sc):
                    ssl = slice(sc * 512, (sc + 1) * 512)
                    t = dramp.tile([6, 512], FP32, name=f"ddena{sc}")
                    nc.sync.dma_start(t[:], stageD[0:1, 0:6, ssl])
                    nc.sync.dma_start(denT[0:6, ssl], t[:])

                def normalize_cc(sc):
                    ssl = slice(sc * 512, (sc + 1) * 512)
                    dden_b = dramp.tile([2, 512], FP32, name=f"ddenb{sc}")
                    nc.sync.dma_start(dden_b[:], stageD[0:1, 6:8, ssl])
                    nc.sync.dma_start(denT[6:8, ssl], dden_b[:])
                    with nc.allow_low_precision(reason="f32r is fp32 bits"):
                        nc.vector.reciprocal(recipT[:, ssl], denT[:, ssl])
                    tmps = []
                    for p in range(NPAIR):
                        bc = bcpsum.tile([128, 512], FP32, tag="bc")
                        nc.tensor.matmul(
                            bc[:], sel_sb[:, p, :], recipT[:HL, ssl],
                            start=True, stop=True,
                        )
                        tmp = smallp.tile([128, 512], F32R, tag=f"tmp{p}")
                        nc.vector.tensor_tensor(
                            tmp[:], numT[:, p, ssl], bc[:], ALU.mult)
                        tmps.append(tmp)
                    zps = bcpsum.tile([DH, 512], FP32, tag="bc", name="zps")
                    for p in range(NPAIR):
                        nc.tensor.matmul(
                            zps[:], fold_sb[:], tmps[p][:],
                            start=(p == 0), stop=(p == NPAIR - 1),
                        )
                    nc.vector.tensor_copy(zsum[:, ssl], zps[:])
                    zin = dramp.tile([DH, 512], BF16, name=f"zin{sc}")
                    nc.sync.dma_start(zin[:], zsum[:, ssl])
                    zout = dramp.tile([DH, 512], BF16, name=f"zout{sc}")
                    nc.gpsimd.collective_compute(
                        "AllReduce", ALU.add, replica_groups=RG,
                        ins=[zin[:].opt()], outs=[zout[:].opt()],
                    )
                    zouts.append(zout)

                GRP = [(p, hh) for p in range(NPAIR) for hh in range(2)]
                # first half: consume prefetched scores; interleave the second
                # half's first score groups so the exp stream never goes cold
                # across the first half's z-chains and CC0's issue window
                sc1_pend = []
                for gi, (p, hh) in enumerate(GRP):
                    emit_z(0, p, hh, w0s[gi])
                    if (p, hh) == (2, 1):
                        den_stage_a(0)
                    if gi in (1, 3):
                        g = GRP[len(sc1_pend)]
                        sc1_pend.append((g[0], g[1], emit_scores(1, *g)))
                normalize_cc(0)
                for (p, hh) in GRP[len(sc1_pend):]:
                    wT = emit_scores(1, p, hh)
                    if len(sc1_pend) >= 2:
                        a = sc1_pend.pop(0)
                        emit_z(1, *a)
                        if a[:2] == (2, 1):
                            den_stage_a(1)
                    sc1_pend.append((p, hh, wT))
                for a in sc1_pend:
                    emit_z(1, *a)
                    if a[:2] == (2, 1):
                        den_stage_a(1)
                normalize_cc(1)

                ff_half(0, rs0)
                ff_half(1, rs1)
                if DBG:
                    nc.sync.dma_start(dq[:], qT[:, 0, :])
                    nc.sync.dma_start(dv[:], vA[:, 0, :, :].rearrange("p h e -> p (h e)"))
                    nc.sync.dma_start(dnum[:], numT[:, 0, :])
                    nc.sync.dma_start(dden[:], denT[:HL, :])
                    nc.sync.dma_start(dzs[:], zsum[:])
                    nc.sync.dma_start(drs[:], rs0[:].bitcast(FP32))
                    nc.sync.dma_start(dsg[:], stageD[:])
                    nc.sync.dma_start(dtm[:], tmps[0][:].bitcast(FP32))

    nc.compile()
    return nc


_NC = None


def _get_nc():
    global _NC
    if _NC is None:
        _NC = build_nc()
    return _NC


def make_in_maps(x, Wq, bq, Wk, bk, Wv, bv, Wf, bf):
    x, Wq, bq, Wk, bk, Wv, bv, Wf, bf = (
        np.asarray(a, dtype=np.float32)
        for a in (x, Wq, bq, Wk, bk, Wv, bv, Wf, bf))

    r = np.arange(128)
    tri = (r[:, None] <= r[None, :]).astype(BF)          # key r allowed for query j
    sel = np.zeros((8, NPAIR, 128), np.float32)
    for p in range(NPAIR):
        for j in range(128):
            sel[2 * p + j // 64, p, j] = 1.0
    wfa = np.concatenate([Wf, bf.reshape(1, D)], axis=0).astype(BF)
    fold = (np.arange(128)[:, None] % 64 == np.arange(DH)[None, :]).astype(np.float32)

    in_maps = []
    for c in range(8):
        b, g = c // 2, c % 2
        hs = slice(g * HL, (g + 1) * HL)
        bqk_l = np.empty((128, 2 * NPAIR), np.float32)
        for p in range(NPAIR):
            bqk_l[:, p] = bq[g * HL + 2 * p: g * HL + 2 * p + 2].reshape(128)
            bqk_l[:, NPAIR + p] = bk[g * HL + 2 * p: g * HL + 2 * p + 2].reshape(128)
        in_maps.append({
            "xT": np.ascontiguousarray(x[b].T).astype(BF),
            "wq": np.ascontiguousarray(
                Wq[hs].transpose(1, 0, 2).reshape(D, HL * DH)).astype(BF),
            "wk": np.ascontiguousarray(
                Wk[hs].transpose(1, 0, 2).reshape(D, HL * DH)).astype(BF),
            "wv": np.ascontiguousarray(
                Wv[hs].transpose(1, 0, 2).reshape(D, HL * DH)).astype(BF),
            "bqk": bqk_l,
            "bvt": np.ascontiguousarray(
                np.broadcast_to(bv[hs].reshape(1, HL * DH), (128, HL * DH))),
            "tri01": tri,
            "onesp": np.ones((128, 8, HL), BF),
            "selp": sel,
            "foldp": fold,
            "wfa": wfa,
            "onesr": np.ones((1, 512), BF),
        })
    return in_maps


def run(in_maps, trace=False, **kw):
    nc = _get_nc()
    return run_bass_kernel_spmd(nc, in_maps, list(range(8)), trace=trace, **kw)


def assemble(results):
    """results: list of 8 per-core dicts -> full [B, S, D] output."""
    out = np.empty((B, S, D), np.float32)
    for b in range(B):
        out[b] = results[2 * b]["out"]
    return out


def bench(in_maps, iters=30, warmup=3):
    """Build the sharded PJRT executable once, run `iters` back-to-back
    executions with device-resident inputs, return (results, per_iter_ns).
    """
    import time

    import jax
    from jax.experimental.shard_map import shard_map
    from jax.sharding import Mesh, PartitionSpec

    from concourse import bass2jax, mybir as _mybir

    nc = _get_nc()
    bass2jax.install_neuronx_cc_hook()
    partition_name = nc.partition_id_tensor.name if nc.partition_id_tensor else None

    in_names, out_names, out_avals, zero_outs = [], [], [], []
    for alloc in nc.m.functions[0].allocations:
        if not isinstance(alloc, _mybir.MemoryLocationSet):
            continue
        name = alloc.memorylocations[0].name
        if alloc.kind == "ExternalInput":
            if name != partition_name:
                in_names.append(name)
        elif alloc.kind == "ExternalOutput":
            out_names.append(name)
            shape = tuple(alloc.tensor_shape)
            dtype = _mybir.dt.np(alloc.dtype)
            out_avals.append(jax.core.ShapedArray(shape, dtype))
            zero_outs.append(np.zeros(shape, dtype))
    n_params = len(in_names)

    all_in_names = list(in_names) + list(out_names)
    if partition_name is not None:
        all_in_names.append(partition_name)

    def _body2(*args):
        operands = list(args)
        if partition_name is not None:
            operands.append(bass2jax.partition_id_tensor())
        outs = bass2jax._bass_exec_p.bind(
            *operands,
            out_avals=tuple(out_avals),
            in_names=tuple(all_in_names),
            out_names=tuple(out_names),
            lowering_input_output_aliases=(),
            sim_require_finite=True,
            sim_require_nnan=True,
            nc=nc,
        )
        return tuple(outs)

    n_cores = 8
    devices = jax.devices()[:n_cores]
    mesh = Mesh(np.asarray(devices), ("core",))
    n_outs = len(out_names)
    sharded = jax.jit(
        shard_map(_body2, mesh=mesh,
                  in_specs=(PartitionSpec("core"),) * (n_params + n_outs),
                  out_specs=(PartitionSpec("core"),) * n_outs,
                  check_rep=False),
        keep_unused=True,
    )
    per_core = [[np.asarray(m[name]) for name in in_names] for m in in_maps]
    concat_in = [np.concatenate([per_core[c][i] for c in range(n_cores)], axis=0)
                 for i in range(n_params)]
    concat_zeros = [np.zeros((n_cores * z.shape[0], *z.shape[1:]), z.dtype)
                    for z in zero_outs]
    dev_in = [jax.device_put(a) for a in concat_in]
    dev_zero = [jax.device_put(a) for a in concat_zeros]

    out_arrs = jax.block_until_ready(sharded(*dev_in, *dev_zero))
    for _ in range(warmup - 1):
        out_arrs = jax.block_until_ready(sharded(*dev_in, *dev_zero))
    t0 = time.perf_counter()
    for _ in range(iters):
        out_arrs = sharded(*dev_in, *dev_zero)
    jax.block_until_ready(out_arrs)
    t1 = time.perf_counter()
    per_iter_ns = (t1 - t0) / iters * 1e9

    results = [
        {name: np.asarray(out_arrs[i]).reshape(n_cores, *out_avals[i].shape)[c]
         for i, name in enumerate(out_names)}
        for c in range(n_cores)
    ]
    return results, per_iter_ns


def kernel(x, Wq, bq, Wk, bk, Wv, bv, Wf, bf):
    in_maps = make_in_maps(x, Wq, bq, Wk, bk, Wv, bv, Wf, bf)
    res = run(in_maps)
    return assemble(res.results)


if __name__ == "__main__":
    nc = build_nc()
    print("build OK")
